# revision 57
# baseline (speedup 1.0000x reference)
"""Cross-attention kernel for TRN2, data-parallel over batch (B=8) on 8 cores.

Reference computation per batch element:
    xt  = proj_in(x)              # [L=4096, E=512], 1x1 conv == matmul
    Q   = xt @ W_q.T + b_q
    K   = ctx @ W_k.T + b_k       # ctx: [S=1024, E]
    V   = ctx @ W_v.T + b_v
    att = softmax(Q @ K.T * scale)
    out = proj_out((att @ V).T)   # [C=512, 64, 64]

Host-side algebraic folds (weights only, exact up to fp rounding):
  * scale, W_pi, W_q, W_k fold into G = (scale * W_q @ W_pi).T @ W_k, so
    logits.T = (G.T-contract ctx).T-contract X.
  * W_v and W_po fold:  WV = (W_po @ W_v).T ; b_o = b_po + W_po @ b_v
  * Vbar[o] = sum_j VW[j, o] = (ctx.sum over keys) @ WV  -- the softmax
    mean-numerator -- is a tiny per-core [512] vector, computed on host.

The two big attention GEMMs (logits ST = GC.T-c X and output U = VW.T-c PT)
run as fp8e4m3 DoubleRow matmuls: 256 contraction rows per instruction --
2x the bf16 MAC rate.  fp8's 3-bit mantissa cannot represent softmax
weights P ~= 1 +- 0.1 (quantization step 0.125 at 1.0), so the softmax is
mean-split:  P = 1 + p,  U = Vbar + sum_j p_j VW_j with p = exp(s) - 1
cast to fp8 (full relative resolution on the deviation).  The Vbar mean
term and the invz division both fold into one DVE scalar_tensor_tensor:
y = (U_dev + Vbar) * invz.

Z = 1024 + sum_j p_j comes from a DoubleRow ones-matmul whose lhsT has 128
ones columns, so the psum holds 128 identical Z rows -- the reciprocal is
then already partition-replicated and no gpsimd broadcast is needed.

The per-core precomputes GC = G.T-c ctx and VW = ctx.T-c WV also run as
fp8 DoubleRow matmuls (gt/ctx/wv arrive as scaled e4m3); the psum->fp8
casts run on the Scalar engine (idle during that phase) with the inverse
input scales folded into the activation scale.  Scales kg/kv are chosen
from exact host-side maxima so GCD/VW8 use the full e4m3 normal range.

A burst of tiny warm-up matmuls on a memset tile runs during the initial
DMA wait so the PE's HAM clock gate (cold = 1.2 GHz, warm = 2.4 GHz) is
already released when the first real matmul issues.
"""

import numpy as np
import ml_dtypes

import concourse.bass as bass
import concourse.mybir as mybir
import concourse.tile as tile
from concourse import bacc
from concourse.bass_utils import run_bass_kernel_spmd

F32 = mybir.dt.float32
BF16 = mybir.dt.bfloat16
F8 = mybir.dt.float8e4
EXP = mybir.ActivationFunctionType.Exp
IDENT = mybir.ActivationFunctionType.Identity
COPY = mybir.ActivationFunctionType.Copy
DR = mybir.MatmulPerfMode.DoubleRow
AXX = mybir.AxisListType.X
ADD = mybir.AluOpType.add
MUL = mybir.AluOpType.mult

NP_F8 = ml_dtypes.float8_e4m3
NP_BF = ml_dtypes.bfloat16

C = 512       # in channels
E = 512       # emb dim
L = 4096      # query length (64*64)
S = 1024      # key length (32*32)
LI = 512      # i-chunk (query) tile size
NCHUNK = L // LI
NCORES = 8
NWARM = 30    # PE warm-up matmuls (N=128) during the DMA lead-in

TRACE = False           # test harness can flip this before calling kernel()
LAST_RESULTS = None     # stashed BassKernelResults for the test harness

_PROGRAM_CACHE = {}


def _two(ap, inner):
    """[128, 2*inner] AP -> [128, 2, inner] for DoubleRow operands."""
    return ap.rearrange("p (two n) -> p two n", two=2, n=inner)


def _build_program(has_q0: bool, has_bo: bool, kg: int, kc: int, kw: int, kv: int):
    nc = bacc.Bacc(
        "TRN2",
        target_bir_lowering=False,
        debug=False,
        enable_asserts=False,
        num_devices=NCORES,
    )
    # x host-permuted to x_d[p, ic*2048 + t*512 + i] = x[t*128+p, ic*512+i]
    # so each chunk load is one contiguous 2KB-per-partition DMA.
    x_d = nc.dram_tensor("x", [128, 4 * L], F8, kind="ExternalInput").ap()
    # ctx pre-scaled by 2^kc into e4m3 normal range, host-permuted et-major:
    # ctx_d[p, et*S + j] = (2^kc ctx)[et*128+p, j]
    ctx_d = nc.dram_tensor("ctx", [128, 4 * S], F8, kind="ExternalInput").ap()
    # gt arrives host-permuted into ct-major blocks: gt_d[p, ct*512+et*128+c']
    # = (2^kg * G.T)[et*128+p, ct*128+c'], e4m3.
    gt_d = nc.dram_tensor("gt", [128, 4 * C], F8, kind="ExternalInput").ap()
    # wv[p, et*E + o] = (2^kw * WV)[et*128+p, o], e4m3 (et-major blocks).
    wv_d = nc.dram_tensor("wv", [128, 4 * E], F8, kind="ExternalInput").ap()
    # vbar_d[p, ot] = (2^kv * Vbar)[ot*128+p]
    vbar_d = nc.dram_tensor("vbar", [128, 4], F32, kind="ExternalInput").ap()
    q0_d = bo_d = None
    if has_q0:
        q0_d = nc.dram_tensor("q0", [128, 8], F32, kind="ExternalInput").ap()
    if has_bo:
        bo_d = nc.dram_tensor("bo", [128, 4], F32, kind="ExternalInput").ap()
    # y device layout mirrors x: y_d[p, ic*2048 + t*512 + i] = y[t*128+p, ...]
    y_d = nc.dram_tensor("y", [128, 4 * L], BF16, kind="ExternalOutput").ap()

    exp_scale = float(2.0 ** -kg)
    gcd_scale = float(2.0 ** -kc)
    vw8_scale = float(2.0 ** (kv - kc - kw))
    zr_scale = float(2.0 ** kv)
    zr_bias = float(S) * zr_scale

    with tile.TileContext(nc) as tc:
        from contextlib import ExitStack

        with ExitStack() as ctx:
            cpool = ctx.enter_context(tc.tile_pool(name="consts", bufs=1))
            ps_s = ctx.enter_context(tc.tile_pool(name="ps_s", bufs=4, space="PSUM"))
            ps_z = ctx.enter_context(tc.tile_pool(name="ps_z", bufs=1, space="PSUM"))
            ps_u = ctx.enter_context(tc.tile_pool(name="ps_u", bufs=3, space="PSUM"))
            xpool = ctx.enter_context(tc.tile_pool(name="xp", bufs=2))
            ppool = ctx.enter_context(tc.tile_pool(name="pp", bufs=2))
            p8pool = ctx.enter_context(tc.tile_pool(name="p8p", bufs=2))
            opool = ctx.enter_context(tc.tile_pool(name="op", bufs=2))
            zpool = ctx.enter_context(tc.tile_pool(name="zp", bufs=2))

            # ---- PE warm-up: release the HAM clock gate during DMA wait ----
            # N=128 matmuls (~107ns cold) keep the PE duty cycle high enough
            # for the HAM activity window to flip to 2.4 GHz before the first
            # real matmul; N=32 warm-ups were too short to register as busy.
            # warm is memset on GpSimd, which is already active running the
            # framework's const-AP memsets -- the tile is ready before the
            # Tensor engine enters main, so warm-up starts immediately
            warm = cpool.tile([128, 128], BF16, name="warm")
            nc.gpsimd.memset(warm[:], 0.0)
            zrb = cpool.tile([128, 1], F32, name="zrb")
            nc.vector.memset(zrb[:], zr_bias)
            wps = ps_z.tile([128, 128], F32, name="wps", tag="z")
            for _ in range(NWARM):
                nc.tensor.matmul(wps[:], warm[:], warm[:], start=True, stop=True)

            # ---- loads in latency-priority order ---------------------------
            # (each dma_start costs ~0.7us of Sync-sequencer descriptor-gen
            # and all tensors are host-permuted to one contiguous descriptor
            # per partition row, so the preload is one DMA per tensor)
            # ctx is jh-major on host, so each half is contiguous and the
            # jh=0 GC groups are fully served by the first 256KB transfer
            GTS = cpool.tile([128, 4 * C], F8, name="gstk", tag="gstk")
            CTXT = cpool.tile([128, 4 * S], F8, name="cstk", tag="cstk")
            nc.sync.dma_start(CTXT[:, 0:2 * S], ctx_d[:, 0:2 * S])
            nc.sync.dma_start(GTS[:], gt_d[:, :])
            nc.sync.dma_start(CTXT[:, 2 * S:4 * S], ctx_d[:, 2 * S:4 * S])
            WVT = cpool.tile([128, 4 * E], F8, name="wstk", tag="wstk")
            nc.sync.dma_start(WVT[:], wv_d[:, :])
            one8_s = cpool.tile([128, 256], F8, name="one8s")
            nc.vector.memset(one8_s[:], 1.0)

            def load_x(ic):
                xt = xpool.tile([128, 4 * LI], F8, name="xc", tag="x")
                nc.sync.dma_start(xt[:], x_d[:, bass.ts(ic, 4 * LI)])
                return xt

            vbar_s = cpool.tile([128, 4], F32, name="vbars")
            nc.sync.dma_start(vbar_s[:], vbar_d[:, :])
            X0 = load_x(0)                                        # prefetch chunk 0
            q0_s = bo_s = None
            if has_q0:
                q0_s = cpool.tile([128, 8], F32, name="q0s")
                nc.sync.dma_start(q0_s[:], q0_d[:, :])
            if has_bo:
                bo_s = cpool.tile([128, 4], F32, name="bos")
                nc.sync.dma_start(bo_s[:], bo_d[:, :])

            # ---- GC[c, j] = sum_e G[c, e] ctx[e, j]  (fp8 DR, once) -------
            # Output goes straight to the DoubleRow-interleaved fp8 layout:
            # GCD[cp][p, jt*256 + t*128 + m] = GC[(2cp+t)*128+p, jt*128+m],
            # scaled 2^kg (the 2^kc input scale divides out in the cast).
            GCD = [
                cpool.tile([128, 2048], F8, name=f"gcd{cp}", tag=f"gcd{cp}")
                for cp in range(2)
            ]
            for jh in range(2):
                # ctx_h[p, jh*2048 + et*512 + j'] = (2^kc ctx)[et*128+p, jh*512+j']
                ctxjh = CTXT[:, jh * 2 * S:(jh + 1) * 2 * S].rearrange(
                    "p (et j) -> p et j", et=4
                )
                for ct in range(4):
                    gps = ps_s.tile([128, LI], F32, name="gps", tag="s")
                    for eh in range(2):
                        nc.tensor.matmul(
                            gps[:],
                            _two(GTS[:, ct * 512 + eh * 256: ct * 512 + (eh + 1) * 256], 128),
                            ctxjh[:, 2 * eh:2 * eh + 2, :],
                            start=(eh == 0),
                            stop=(eh == 1),
                            perf_mode=DR,
                        )
                    dst = GCD[ct // 2][:, jh * 1024:(jh + 1) * 1024].rearrange(
                        "p (j two m) -> p j two m", two=2, m=128
                    )[:, :, ct % 2, :]
                    # alternate casts between Scalar and DVE: a single cast
                    # stream recycles the 4-deep psum pool slower than the
                    # matmuls fill it and throttles the whole GC/VW phase
                    if ct % 2 == 0:
                        nc.scalar.activation(
                            dst, gps[:].rearrange("p (j m) -> p j m", m=128),
                            COPY, scale=gcd_scale,
                        )
                    else:
                        nc.vector.tensor_scalar_mul(
                            dst, gps[:].rearrange("p (j m) -> p j m", m=128),
                            gcd_scale,
                        )

            # ---- VW[j, o] = sum_e ctx[e, j] WV[e, o]  (fp8 DR, once) ------
            # VW8D[jp][p, t*E + o] = 2^kv * VW[(2jp+t)*128+p, o]
            VW8D = [None] * 4

            def vw_group(jt):
                ctxjh = CTXT[:, (jt // 4) * 2 * S:(jt // 4 + 1) * 2 * S].rearrange(
                    "p (et j) -> p et j", et=4
                )
                js = jt % 4
                vps = ps_s.tile([128, E], F32, name="vps", tag="s")
                for eh in range(2):
                    nc.tensor.matmul(
                        vps[:],
                        ctxjh[:, 2 * eh:2 * eh + 2, js * 128:(js + 1) * 128],
                        _two(WVT[:, eh * 2 * E:(eh + 1) * 2 * E], E),
                        start=(eh == 0),
                        stop=(eh == 1),
                        perf_mode=DR,
                    )
                jp, t = jt // 2, jt % 2
                if t == 0:
                    VW8D[jp] = cpool.tile(
                        [128, 2 * E], F8, name=f"vw8_{jp}", tag=f"vw8_{jp}"
                    )
                if jt % 2 == 0:
                    nc.scalar.activation(
                        VW8D[jp][:, t * E:(t + 1) * E], vps[:], COPY,
                        scale=vw8_scale,
                    )
                else:
                    nc.vector.tensor_scalar_mul(
                        VW8D[jp][:, t * E:(t + 1) * E], vps[:], vw8_scale
                    )

            def st_group(ic, jt, X, pcur, p8cur):
                """ST[j,i] for one j-tile: 2 DoubleRow fp8 matmuls, exp on
                scalar (with the 2^-kg descale); after each jt-pair completes
                one DVE op casts p8 = P - 1 for the whole pair."""
                sps = ps_s.tile([128, LI], F32, name="sps", tag="s")
                nc.tensor.matmul(
                    sps[:],
                    _two(GCD[0][:, jt * 256:(jt + 1) * 256], 128),
                    _two(X[:, 0:2 * LI], LI),
                    start=True,
                    stop=False,
                    perf_mode=DR,
                )
                nc.tensor.matmul(
                    sps[:],
                    _two(GCD[1][:, jt * 256:(jt + 1) * 256], 128),
                    _two(X[:, 2 * LI:4 * LI], LI),
                    start=False,
                    stop=True,
                    perf_mode=DR,
                )
                jp, t = jt // 2, jt % 2
                if t == 0:
                    pcur[jp] = ppool.tile(
                        [128, 2 * LI], BF16, name=f"pt{jp}", tag=f"p{jp}"
                    )
                p = pcur[jp]
                if has_q0:
                    nc.scalar.activation(
                        p[:, t * LI:(t + 1) * LI], sps[:], EXP,
                        bias=q0_s[:, jt:jt + 1], scale=exp_scale,
                    )
                else:
                    nc.scalar.activation(
                        p[:, t * LI:(t + 1) * LI], sps[:], EXP, scale=exp_scale
                    )
                if t == 1:
                    p8cur[jp] = p8pool.tile(
                        [128, 2 * LI], F8, name=f"p8_{jp}", tag=f"p8_{jp}"
                    )
                    nc.vector.tensor_scalar_add(p8cur[jp][:], p[:], -1.0)

            def zsum_emit(p8prev):
                """2^kv * Z rows (all 128 partitions identical) via DoubleRow
                ones-matmul; zr = 2^kv*(1024 + sum p) on Scalar, reciprocal
                on DVE -- already partition-replicated, no broadcast."""
                zps = ps_z.tile([128, LI], F32, name="zps", tag="z")
                for jp in range(4):
                    nc.tensor.matmul(
                        zps[:],
                        _two(one8_s[:], 128),
                        _two(p8prev[jp][:], LI),
                        start=(jp == 0),
                        stop=(jp == 3),
                        perf_mode=DR,
                    )
                zr = zpool.tile([128, LI], F32, name="zr", tag="zr")
                nc.scalar.activation(zr[:], zps[:], IDENT, bias=zrb[:, 0:1], scale=zr_scale)
                invz = zpool.tile([128, LI], F32, name="invz", tag="invz")
                nc.vector.reciprocal_approx_fast(out=invz[:], in_=zr[:])
                return invz

            def u_group(ic, ot, p8prev, invz, ostate):
                """U_dev[o,i] = sum_j p_j VW[j,o] (psum, scaled 2^kv), then
                y = (U_dev + 2^kv Vbar) * invz on DVE (bf16 out).  The four
                ot tiles share one SBUF tile and ship as a single DMA."""
                if ic == NCHUNK - 1 and ot == 3:
                    # the Z bank is free after the final zsum; using it here
                    # avoids the last U group stalling on the ps_u rotation
                    ups = ps_z.tile([128, LI], F32, name="upz", tag="z")
                else:
                    ups = ps_u.tile([128, LI], F32, name="ups", tag="u")
                for jp in range(4):
                    nc.tensor.matmul(
                        ups[:],
                        _two(VW8D[jp][:], E)[:, :, ot * 128:(ot + 1) * 128],
                        _two(p8prev[jp][:], LI),
                        start=(jp == 0),
                        stop=(jp == 3),
                        perf_mode=DR,
                    )
                if ot == 0:
                    ostate["o"] = opool.tile([128, 4 * LI], BF16, name="ot", tag="o")
                o = ostate["o"]
                nc.vector.scalar_tensor_tensor(
                    o[:, ot * LI:(ot + 1) * LI], ups[:],
                    vbar_s[:, ot:ot + 1], invz[:], ADD, MUL,
                )
                if has_bo:
                    nc.vector.tensor_scalar_add(
                        o[:, ot * LI:(ot + 1) * LI],
                        o[:, ot * LI:(ot + 1) * LI], bo_s[:, ot:ot + 1],
                    )
                if ic == NCHUNK - 1:
                    # final chunk: ship each ot slice as soon as it is ready
                    # so the kernel's last DMA isn't gated on all four
                    nc.sync.dma_start(
                        y_d[:, ic * 4 * LI + ot * LI: ic * 4 * LI + (ot + 1) * LI],
                        o[:, ot * LI:(ot + 1) * LI],
                    )
                elif ot == 3:
                    nc.sync.dma_start(y_d[:, bass.ts(ic, 4 * LI)], o[:])

            # ---- window 0: ST(0) interleaved with the VW precompute -------
            # the exp pipeline starts while VW still owns the PE, so the
            # tail ST(0) groups aren't throttled to the Scalar exp rate
            X = X0
            pcur, p8cur = {}, {}
            st_group(0, 0, X, pcur, p8cur)
            st_group(0, 1, X, pcur, p8cur)
            for jt in range(8):
                vw_group(jt)
                if jt < 6:
                    st_group(0, jt + 2, X, pcur, p8cur)
            Xnext = load_x(1)

            # ---- windows 1..8: ST(w) interleaved with U(w-1) --------------
            invz_next = None
            for w in range(1, NCHUNK + 1):
                p8prev, p8cur = p8cur, {}
                pcur = {}
                X, Xnext = Xnext, (load_x(w + 1) if w + 1 < NCHUNK else None)
                invz = invz_next
                invz_next = None
                ostate = {}
                # in the final ST window, front-load the last jt pairs so the
                # final chunk's zsum (emitted at k=3 of THIS window, hidden
                # behind the U groups) isn't gated on the very last exp/cast
                pairs_at_k = [2, 4, 2, 0] if w == NCHUNK - 1 else [2, 2, 2, 2]
                jt_next = 0
                for k in range(4):
                    if w < NCHUNK:
                        for _ in range(pairs_at_k[k]):
                            st_group(w, jt_next, X, pcur, p8cur)
                            jt_next += 1
                    if k == 0 and invz is None:
                        invz = zsum_emit(p8prev)
                    if k == 3 and w == NCHUNK - 1:
                        # final chunk's Z overlaps the last U group of the
                        # previous chunk; its invz is ready when window
                        # NCHUNK's first U group stops
                        invz_next = zsum_emit(p8cur)
                    u_group(w - 1, k, p8prev, invz, ostate)

    nc.compile()
    return nc


def kernel(**inputs) -> np.ndarray:
    global LAST_RESULTS
    x = np.asarray(inputs["x"], dtype=np.float32)
    context = np.asarray(inputs["context"], dtype=np.float32)
    W_pi = np.asarray(inputs["W_pi"], dtype=np.float64)
    b_pi = np.asarray(inputs["b_pi"], dtype=np.float64)
    W_q = np.asarray(inputs["W_q"], dtype=np.float64)
    b_q = np.asarray(inputs["b_q"], dtype=np.float64)
    W_k = np.asarray(inputs["W_k"], dtype=np.float64)
    W_v = np.asarray(inputs["W_v"], dtype=np.float64)
    b_v = np.asarray(inputs["b_v"], dtype=np.float64)
    W_po = np.asarray(inputs["W_po"], dtype=np.float64)
    b_po = np.asarray(inputs["b_po"], dtype=np.float64)

    scale = float(E) ** -0.5
    Wqpi = scale * (W_q @ W_pi)                            # [dq, c]
    G = (Wqpi.T @ W_k)                                     # [c, e]
    b_row = scale * (W_q @ b_pi + b_q)
    q0_e = (W_k.T @ b_row).astype(np.float64)              # [e]
    WV64 = (W_po @ W_v).T                                  # [e, o]
    b_o = (b_po + W_po @ b_v).astype(np.float32)           # [o]

    ctx_all = context.reshape(NCORES, E, S)
    G32 = G.astype(np.float32)
    # exact per-core maxima for the fp8 scale choices
    gc_max = 1e-30
    vw_max = 1e-30
    ctx_max = float(np.abs(ctx_all).max())
    WV32 = WV64.astype(np.float32)
    for c in range(NCORES):
        gc_max = max(gc_max, float(np.abs(G32 @ ctx_all[c]).max()))
        vw_max = max(vw_max, float(np.abs(ctx_all[c].T @ WV32).max()))
    kc = int(np.floor(np.log2(200.0 / ctx_max)))
    kw = int(np.floor(np.log2(200.0 / max(float(np.abs(WV64).max()), 1e-30))))
    kg = int(np.floor(np.log2(200.0 / gc_max)))
    kv = int(np.floor(np.log2(200.0 / vw_max)))

    # TRN e4m3 tops out at +-240 (S.1111.000 is inf), so clip before casting.
    GT = np.clip(G.T * (2.0 ** kg), -240.0, 240.0).astype(np.float32)   # [e, c]
    # ct-major block permutation: A[p, ct*512+et*128+c'] = GT[et*128+p, ct*128+c']
    GT = np.ascontiguousarray(
        GT.reshape(4, 128, 4, 128).transpose(1, 2, 0, 3).reshape(128, 4 * C)
    ).astype(NP_F8)
    # wv et-major blocks: [p, et*E + o] = 2^kw WV[et*128+p, o]
    WVS = np.ascontiguousarray(
        np.clip(WV64 * (2.0 ** kw), -240.0, 240.0).astype(np.float32)
        .reshape(4, 128, E).transpose(1, 0, 2).reshape(128, 4 * E)
    ).astype(NP_F8)

    has_q0 = bool(np.any(q0_e))
    has_bo = bool(np.any(b_o))
    key = (has_q0, has_bo, kg, kc, kw, kv)
    if key not in _PROGRAM_CACHE:
        _PROGRAM_CACHE[key] = _build_program(has_q0, has_bo, kg, kc, kw, kv)
    nc = _PROGRAM_CACHE[key]

    in_maps = []
    for c in range(NCORES):
        ctx_mat = ctx_all[c]
        vbar = (ctx_mat.sum(axis=1).astype(np.float64) @ WV64) * (2.0 ** kv)
        # x permuted so chunk loads are contiguous per partition:
        # x_h[p, ic*2048 + t*512 + i] = x[t*128+p, ic*512+i]
        x8 = x[c].reshape(C, L).astype(NP_F8)
        x_h = np.ascontiguousarray(
            x8.reshape(4, 128, NCHUNK, LI).transpose(1, 2, 0, 3).reshape(128, 4 * L)
        )
        # ctx permuted jh-major then et-major:
        # ctx_h[p, jh*2048 + et*512 + j'] = (2^kc ctx)[et*128+p, jh*512+j']
        ctx8 = np.clip(ctx_mat * (2.0 ** kc), -240.0, 240.0).astype(NP_F8)
        ctx_h = np.ascontiguousarray(
            ctx8.reshape(4, 128, 2, 512).transpose(1, 2, 0, 3).reshape(128, 4 * S)
        )
        m = {
            "x": x_h,
            "ctx": ctx_h,
            "gt": GT,
            "wv": WVS,
            "vbar": np.ascontiguousarray(
                vbar.astype(np.float32).reshape(4, 128).T
            ),
        }
        if has_q0:
            # logits bias per key j: q0_e . ctx[:, j]  -> [S] -> [128, 8]
            q0j = (q0_e @ ctx_mat.astype(np.float64)).astype(np.float32)
            m["q0"] = np.ascontiguousarray(q0j.reshape(8, 128).T)
        if has_bo:
            m["bo"] = np.ascontiguousarray(b_o.reshape(4, 128).T)
        in_maps.append(m)

    res = run_bass_kernel_spmd(nc, in_maps, core_ids=list(range(NCORES)), trace=TRACE)
    LAST_RESULTS = res
    # y_h[p, ic*2048 + t*512 + i] = y[t*128+p, ic*512+i] -- invert the permute
    y = np.stack(
        [
            np.asarray(res.results[c]["y"]).astype(np.float32)
            .reshape(128, NCHUNK, 4, LI).transpose(2, 0, 1, 3).reshape(C, L)
            for c in range(NCORES)
        ],
        axis=0,
    )
    return np.ascontiguousarray(y.reshape(NCORES, C, 64, 64))


# revision 59
# speedup vs baseline: 29020.2264x; 29020.2264x over previous
"""Cross-attention kernel for TRN2, data-parallel over batch (B=8) on 8 cores.

Reference computation per batch element:
    xt  = proj_in(x)              # [L=4096, E=512], 1x1 conv == matmul
    Q   = xt @ W_q.T + b_q
    K   = ctx @ W_k.T + b_k       # ctx: [S=1024, E]
    V   = ctx @ W_v.T + b_v
    att = softmax(Q @ K.T * scale)
    out = proj_out((att @ V).T)   # [C=512, 64, 64]

Host-side algebraic folds (weights only, exact up to fp rounding):
  * scale, W_pi, W_q, W_k fold into G = (scale * W_q @ W_pi).T @ W_k, so
    logits.T = (G.T-contract ctx).T-contract X.
  * W_v and W_po fold:  WV = (W_po @ W_v).T ; b_o = b_po + W_po @ b_v
  * Vbar[o] = sum_j VW[j, o] = (ctx.sum over keys) @ WV  -- the softmax
    mean-numerator -- is a tiny per-core [512] vector, computed on host.

The two big attention GEMMs (logits ST = GC.T-c X and output U = VW.T-c PT)
run as fp8e4m3 DoubleRow matmuls: 256 contraction rows per instruction --
2x the bf16 MAC rate.  fp8's 3-bit mantissa cannot represent softmax
weights P ~= 1 +- 0.1 (quantization step 0.125 at 1.0), so the softmax is
mean-split:  P = 1 + p,  U = Vbar + sum_j p_j VW_j with p = exp(s) - 1
cast to fp8 (full relative resolution on the deviation).  The Vbar mean
term and the invz division both fold into one DVE scalar_tensor_tensor:
y = (U_dev + Vbar) * invz.

Z = 1024 + sum_j p_j comes from a DoubleRow ones-matmul whose lhsT has 128
ones columns, so the psum holds 128 identical Z rows -- the reciprocal is
then already partition-replicated and no gpsimd broadcast is needed.

The per-core precomputes GC = G.T-c ctx and VW = ctx.T-c WV also run as
fp8 DoubleRow matmuls (gt/ctx/wv arrive as scaled e4m3); the psum->fp8
casts run on the Scalar engine (idle during that phase) with the inverse
input scales folded into the activation scale.  Scales kg/kv are chosen
from exact host-side maxima so GCD/VW8 use the full e4m3 normal range.

A burst of tiny warm-up matmuls on a memset tile runs during the initial
DMA wait so the PE's HAM clock gate (cold = 1.2 GHz, warm = 2.4 GHz) is
already released when the first real matmul issues.
"""

import numpy as np
import ml_dtypes

import concourse.bass as bass
import concourse.mybir as mybir
import concourse.tile as tile
from concourse import bacc
from concourse.bass_utils import run_bass_kernel_spmd

F32 = mybir.dt.float32
BF16 = mybir.dt.bfloat16
F8 = mybir.dt.float8e4
EXP = mybir.ActivationFunctionType.Exp
IDENT = mybir.ActivationFunctionType.Identity
COPY = mybir.ActivationFunctionType.Copy
DR = mybir.MatmulPerfMode.DoubleRow
AXX = mybir.AxisListType.X
ADD = mybir.AluOpType.add
MUL = mybir.AluOpType.mult

NP_F8 = ml_dtypes.float8_e4m3
NP_BF = ml_dtypes.bfloat16

C = 512       # in channels
E = 512       # emb dim
L = 4096      # query length (64*64)
S = 1024      # key length (32*32)
LI = 512      # i-chunk (query) tile size
NCHUNK = L // LI
NCORES = 8
NWARM = 28    # PE warm-up matmuls (N=128) during the DMA lead-in

TRACE = False           # test harness can flip this before calling kernel()
LAST_RESULTS = None     # stashed BassKernelResults for the test harness

_PROGRAM_CACHE = {}


def _two(ap, inner):
    """[128, 2*inner] AP -> [128, 2, inner] for DoubleRow operands."""
    return ap.rearrange("p (two n) -> p two n", two=2, n=inner)


def _build_program(has_q0: bool, has_bo: bool, kg: int, kc: int, kw: int, kv: int):
    nc = bacc.Bacc(
        "TRN2",
        target_bir_lowering=False,
        debug=False,
        enable_asserts=False,
        num_devices=NCORES,
    )
    # x host-permuted to x_d[p, ic*2048 + t*512 + i] = x[t*128+p, ic*512+i]
    # so each chunk load is one contiguous 2KB-per-partition DMA.
    x_d = nc.dram_tensor("x", [128, 4 * L], F8, kind="ExternalInput").ap()
    # ctx pre-scaled by 2^kc into e4m3 normal range, host-permuted et-major:
    # ctx_d[p, et*S + j] = (2^kc ctx)[et*128+p, j]
    ctx_d = nc.dram_tensor("ctx", [128, 4 * S], F8, kind="ExternalInput").ap()
    # gt arrives host-permuted into ct-major blocks: gt_d[p, ct*512+et*128+c']
    # = (2^kg * G.T)[et*128+p, ct*128+c'], e4m3.
    gt_d = nc.dram_tensor("gt", [128, 4 * C], F8, kind="ExternalInput").ap()
    # wv[p, et*E + o] = (2^kw * WV)[et*128+p, o], e4m3 (et-major blocks).
    wv_d = nc.dram_tensor("wv", [128, 4 * E], F8, kind="ExternalInput").ap()
    # vbar_d[p, ot] = (2^kv * Vbar)[ot*128+p]
    vbar_d = nc.dram_tensor("vbar", [128, 4], F32, kind="ExternalInput").ap()
    q0_d = bo_d = None
    if has_q0:
        q0_d = nc.dram_tensor("q0", [128, 8], F32, kind="ExternalInput").ap()
    if has_bo:
        bo_d = nc.dram_tensor("bo", [128, 4], F32, kind="ExternalInput").ap()
    # y device layout mirrors x: y_d[p, ic*2048 + t*512 + i] = y[t*128+p, ...]
    y_d = nc.dram_tensor("y", [128, 4 * L], BF16, kind="ExternalOutput").ap()

    exp_scale = float(2.0 ** -kg)
    gcd_scale = float(2.0 ** -kc)
    vw8_scale = float(2.0 ** (kv - kc - kw))
    zr_scale = float(2.0 ** kv)
    zr_bias = float(S) * zr_scale

    with tile.TileContext(nc) as tc:
        from contextlib import ExitStack

        with ExitStack() as ctx:
            cpool = ctx.enter_context(tc.tile_pool(name="consts", bufs=1))
            ps_s = ctx.enter_context(tc.tile_pool(name="ps_s", bufs=4, space="PSUM"))
            ps_z = ctx.enter_context(tc.tile_pool(name="ps_z", bufs=1, space="PSUM"))
            ps_u = ctx.enter_context(tc.tile_pool(name="ps_u", bufs=3, space="PSUM"))
            xpool = ctx.enter_context(tc.tile_pool(name="xp", bufs=2))
            ppool = ctx.enter_context(tc.tile_pool(name="pp", bufs=2))
            p8pool = ctx.enter_context(tc.tile_pool(name="p8p", bufs=2))
            opool = ctx.enter_context(tc.tile_pool(name="op", bufs=2))
            zpool = ctx.enter_context(tc.tile_pool(name="zp", bufs=2))

            # ---- PE warm-up: release the HAM clock gate during DMA wait ----
            # N=128 matmuls (~107ns cold) keep the PE duty cycle high enough
            # for the HAM activity window to flip to 2.4 GHz before the first
            # real matmul; N=32 warm-ups were too short to register as busy.
            # warm is memset on GpSimd, which is already active running the
            # framework's const-AP memsets -- the tile is ready before the
            # Tensor engine enters main, so warm-up starts immediately
            warm = cpool.tile([128, 128], BF16, name="warm")
            nc.gpsimd.memset(warm[:], 0.0)
            zrb = cpool.tile([128, 1], F32, name="zrb")
            nc.vector.memset(zrb[:], zr_bias)
            wps = ps_z.tile([128, 128], F32, name="wps", tag="z")
            for _ in range(NWARM):
                nc.tensor.matmul(wps[:], warm[:], warm[:], start=True, stop=True)

            # ---- loads in latency-priority order ---------------------------
            # (each dma_start costs ~0.7us of Sync-sequencer descriptor-gen
            # and all tensors are host-permuted to one contiguous descriptor
            # per partition row, so the preload is one DMA per tensor)
            # ctx is jh-major and gt ct-major on host, so both split into
            # contiguous halves matching the GC group read order: the first
            # (jh=0, ct=0/1) groups are served by the first two 128-256KB
            # transfers and later halves land while the PE works
            GTS = cpool.tile([128, 4 * C], F8, name="gstk", tag="gstk")
            CTXT = cpool.tile([128, 4 * S], F8, name="cstk", tag="cstk")
            nc.sync.dma_start(CTXT[:, 0:2 * S], ctx_d[:, 0:2 * S])
            nc.sync.dma_start(GTS[:, 0:1024], gt_d[:, 0:1024])
            nc.sync.dma_start(GTS[:, 1024:2048], gt_d[:, 1024:2048])
            nc.sync.dma_start(CTXT[:, 2 * S:4 * S], ctx_d[:, 2 * S:4 * S])
            WVT = cpool.tile([128, 4 * E], F8, name="wstk", tag="wstk")
            nc.sync.dma_start(WVT[:], wv_d[:, :])
            one8_s = cpool.tile([128, 256], F8, name="one8s")
            nc.vector.memset(one8_s[:], 1.0)

            def load_x(ic):
                xt = xpool.tile([128, 4 * LI], F8, name="xc", tag="x")
                nc.sync.dma_start(xt[:], x_d[:, bass.ts(ic, 4 * LI)])
                return xt

            vbar_s = cpool.tile([128, 4], F32, name="vbars")
            nc.sync.dma_start(vbar_s[:], vbar_d[:, :])
            X0 = load_x(0)                                        # prefetch chunk 0
            q0_s = bo_s = None
            if has_q0:
                q0_s = cpool.tile([128, 8], F32, name="q0s")
                nc.sync.dma_start(q0_s[:], q0_d[:, :])
            if has_bo:
                bo_s = cpool.tile([128, 4], F32, name="bos")
                nc.sync.dma_start(bo_s[:], bo_d[:, :])

            # ---- GC[c, j] = sum_e G[c, e] ctx[e, j]  (fp8 DR, once) -------
            # Output goes straight to the DoubleRow-interleaved fp8 layout:
            # GCD[cp][p, jt*256 + t*128 + m] = GC[(2cp+t)*128+p, jt*128+m],
            # scaled 2^kg (the 2^kc input scale divides out in the cast).
            GCD = [
                cpool.tile([128, 2048], F8, name=f"gcd{cp}", tag=f"gcd{cp}")
                for cp in range(2)
            ]
            for jh in range(2):
                # ctx_h[p, jh*2048 + et*512 + j'] = (2^kc ctx)[et*128+p, jh*512+j']
                ctxjh = CTXT[:, jh * 2 * S:(jh + 1) * 2 * S].rearrange(
                    "p (et j) -> p et j", et=4
                )
                for ct in range(4):
                    gps = ps_s.tile([128, LI], F32, name="gps", tag="s")
                    for eh in range(2):
                        nc.tensor.matmul(
                            gps[:],
                            _two(GTS[:, ct * 512 + eh * 256: ct * 512 + (eh + 1) * 256], 128),
                            ctxjh[:, 2 * eh:2 * eh + 2, :],
                            start=(eh == 0),
                            stop=(eh == 1),
                            perf_mode=DR,
                        )
                    dst = GCD[ct // 2][:, jh * 1024:(jh + 1) * 1024].rearrange(
                        "p (j two m) -> p j two m", two=2, m=128
                    )[:, :, ct % 2, :]
                    # alternate casts between Scalar and DVE: a single cast
                    # stream recycles the 4-deep psum pool slower than the
                    # matmuls fill it and throttles the whole GC/VW phase
                    if ct % 2 == 0:
                        nc.scalar.activation(
                            dst, gps[:].rearrange("p (j m) -> p j m", m=128),
                            COPY, scale=gcd_scale,
                        )
                    else:
                        nc.vector.tensor_scalar_mul(
                            dst, gps[:].rearrange("p (j m) -> p j m", m=128),
                            gcd_scale,
                        )

            # ---- VW[j, o] = sum_e ctx[e, j] WV[e, o]  (fp8 DR, once) ------
            # VW8D[jp][p, t*E + o] = 2^kv * VW[(2jp+t)*128+p, o]
            VW8D = [None] * 4

            def vw_group(jt):
                ctxjh = CTXT[:, (jt // 4) * 2 * S:(jt // 4 + 1) * 2 * S].rearrange(
                    "p (et j) -> p et j", et=4
                )
                js = jt % 4
                vps = ps_s.tile([128, E], F32, name="vps", tag="s")
                for eh in range(2):
                    nc.tensor.matmul(
                        vps[:],
                        ctxjh[:, 2 * eh:2 * eh + 2, js * 128:(js + 1) * 128],
                        _two(WVT[:, eh * 2 * E:(eh + 1) * 2 * E], E),
                        start=(eh == 0),
                        stop=(eh == 1),
                        perf_mode=DR,
                    )
                jp, t = jt // 2, jt % 2
                if t == 0:
                    VW8D[jp] = cpool.tile(
                        [128, 2 * E], F8, name=f"vw8_{jp}", tag=f"vw8_{jp}"
                    )
                if jt % 2 == 0:
                    nc.scalar.activation(
                        VW8D[jp][:, t * E:(t + 1) * E], vps[:], COPY,
                        scale=vw8_scale,
                    )
                else:
                    nc.vector.tensor_scalar_mul(
                        VW8D[jp][:, t * E:(t + 1) * E], vps[:], vw8_scale
                    )

            def st_group(ic, jt, X, pcur, p8cur):
                """ST[j,i] for one j-tile: 2 DoubleRow fp8 matmuls, exp on
                scalar (with the 2^-kg descale); after each jt-pair completes
                one DVE op casts p8 = P - 1 for the whole pair."""
                sps = ps_s.tile([128, LI], F32, name="sps", tag="s")
                nc.tensor.matmul(
                    sps[:],
                    _two(GCD[0][:, jt * 256:(jt + 1) * 256], 128),
                    _two(X[:, 0:2 * LI], LI),
                    start=True,
                    stop=False,
                    perf_mode=DR,
                )
                nc.tensor.matmul(
                    sps[:],
                    _two(GCD[1][:, jt * 256:(jt + 1) * 256], 128),
                    _two(X[:, 2 * LI:4 * LI], LI),
                    start=False,
                    stop=True,
                    perf_mode=DR,
                )
                jp, t = jt // 2, jt % 2
                if t == 0:
                    pcur[jp] = ppool.tile(
                        [128, 2 * LI], BF16, name=f"pt{jp}", tag=f"p{jp}"
                    )
                p = pcur[jp]
                if has_q0:
                    nc.scalar.activation(
                        p[:, t * LI:(t + 1) * LI], sps[:], EXP,
                        bias=q0_s[:, jt:jt + 1], scale=exp_scale,
                    )
                else:
                    nc.scalar.activation(
                        p[:, t * LI:(t + 1) * LI], sps[:], EXP, scale=exp_scale
                    )
                if t == 1:
                    p8cur[jp] = p8pool.tile(
                        [128, 2 * LI], F8, name=f"p8_{jp}", tag=f"p8_{jp}"
                    )
                    nc.vector.tensor_scalar_add(p8cur[jp][:], p[:], -1.0)

            def zsum_emit(p8prev):
                """2^kv * Z rows (all 128 partitions identical) via DoubleRow
                ones-matmul; zr = 2^kv*(1024 + sum p) on Scalar, reciprocal
                on DVE -- already partition-replicated, no broadcast."""
                zps = ps_z.tile([128, LI], F32, name="zps", tag="z")
                for jp in range(4):
                    nc.tensor.matmul(
                        zps[:],
                        _two(one8_s[:], 128),
                        _two(p8prev[jp][:], LI),
                        start=(jp == 0),
                        stop=(jp == 3),
                        perf_mode=DR,
                    )
                zr = zpool.tile([128, LI], F32, name="zr", tag="zr")
                nc.scalar.activation(zr[:], zps[:], IDENT, bias=zrb[:, 0:1], scale=zr_scale)
                invz = zpool.tile([128, LI], F32, name="invz", tag="invz")
                nc.vector.reciprocal_approx_fast(out=invz[:], in_=zr[:])
                return invz

            def u_group(ic, ot, p8prev, invz, ostate):
                """U_dev[o,i] = sum_j p_j VW[j,o] (psum, scaled 2^kv), then
                y = (U_dev + 2^kv Vbar) * invz on DVE (bf16 out).  The four
                ot tiles share one SBUF tile and ship as a single DMA."""
                if ic == NCHUNK - 1 and ot == 3:
                    # the Z bank is free after the final zsum; using it here
                    # avoids the last U group stalling on the ps_u rotation
                    ups = ps_z.tile([128, LI], F32, name="upz", tag="z")
                else:
                    ups = ps_u.tile([128, LI], F32, name="ups", tag="u")
                for jp in range(4):
                    nc.tensor.matmul(
                        ups[:],
                        _two(VW8D[jp][:], E)[:, :, ot * 128:(ot + 1) * 128],
                        _two(p8prev[jp][:], LI),
                        start=(jp == 0),
                        stop=(jp == 3),
                        perf_mode=DR,
                    )
                if ot == 0:
                    ostate["o"] = opool.tile([128, 4 * LI], BF16, name="ot", tag="o")
                o = ostate["o"]
                nc.vector.scalar_tensor_tensor(
                    o[:, ot * LI:(ot + 1) * LI], ups[:],
                    vbar_s[:, ot:ot + 1], invz[:], ADD, MUL,
                )
                if has_bo:
                    nc.vector.tensor_scalar_add(
                        o[:, ot * LI:(ot + 1) * LI],
                        o[:, ot * LI:(ot + 1) * LI], bo_s[:, ot:ot + 1],
                    )
                if ic == NCHUNK - 1:
                    # final chunk: ship each ot slice as soon as it is ready
                    # so the kernel's last DMA isn't gated on all four
                    nc.sync.dma_start(
                        y_d[:, ic * 4 * LI + ot * LI: ic * 4 * LI + (ot + 1) * LI],
                        o[:, ot * LI:(ot + 1) * LI],
                    )
                elif ot == 3:
                    nc.sync.dma_start(y_d[:, bass.ts(ic, 4 * LI)], o[:])

            # ---- window 0: ST(0) interleaved with the VW precompute -------
            # the exp pipeline starts while VW still owns the PE, so the
            # tail ST(0) groups aren't throttled to the Scalar exp rate
            X = X0
            pcur, p8cur = {}, {}
            st_group(0, 0, X, pcur, p8cur)
            st_group(0, 1, X, pcur, p8cur)
            for jt in range(8):
                vw_group(jt)
                if jt < 6:
                    st_group(0, jt + 2, X, pcur, p8cur)
            Xnext = load_x(1)

            # ---- windows 1..8: ST(w) interleaved with U(w-1) --------------
            invz_next = None
            for w in range(1, NCHUNK + 1):
                p8prev, p8cur = p8cur, {}
                pcur = {}
                X, Xnext = Xnext, (load_x(w + 1) if w + 1 < NCHUNK else None)
                invz = invz_next
                invz_next = None
                ostate = {}
                # in the final ST window, front-load the last jt pairs so the
                # final chunk's zsum (emitted at k=3 of THIS window, hidden
                # behind the U groups) isn't gated on the very last exp/cast
                pairs_at_k = [2, 4, 2, 0] if w == NCHUNK - 1 else [2, 2, 2, 2]
                jt_next = 0
                for k in range(4):
                    if w < NCHUNK:
                        for _ in range(pairs_at_k[k]):
                            st_group(w, jt_next, X, pcur, p8cur)
                            jt_next += 1
                    if k == 0 and invz is None:
                        invz = zsum_emit(p8prev)
                    if k == 3 and w == NCHUNK - 1:
                        # final chunk's Z overlaps the last U group of the
                        # previous chunk; its invz is ready when window
                        # NCHUNK's first U group stops
                        invz_next = zsum_emit(p8cur)
                    u_group(w - 1, k, p8prev, invz, ostate)

    nc.compile()
    return nc


def kernel(**inputs) -> np.ndarray:
    global LAST_RESULTS
    x = np.asarray(inputs["x"], dtype=np.float32)
    context = np.asarray(inputs["context"], dtype=np.float32)
    W_pi = np.asarray(inputs["W_pi"], dtype=np.float64)
    b_pi = np.asarray(inputs["b_pi"], dtype=np.float64)
    W_q = np.asarray(inputs["W_q"], dtype=np.float64)
    b_q = np.asarray(inputs["b_q"], dtype=np.float64)
    W_k = np.asarray(inputs["W_k"], dtype=np.float64)
    W_v = np.asarray(inputs["W_v"], dtype=np.float64)
    b_v = np.asarray(inputs["b_v"], dtype=np.float64)
    W_po = np.asarray(inputs["W_po"], dtype=np.float64)
    b_po = np.asarray(inputs["b_po"], dtype=np.float64)

    scale = float(E) ** -0.5
    Wqpi = scale * (W_q @ W_pi)                            # [dq, c]
    G = (Wqpi.T @ W_k)                                     # [c, e]
    b_row = scale * (W_q @ b_pi + b_q)
    q0_e = (W_k.T @ b_row).astype(np.float64)              # [e]
    WV64 = (W_po @ W_v).T                                  # [e, o]
    b_o = (b_po + W_po @ b_v).astype(np.float32)           # [o]

    ctx_all = context.reshape(NCORES, E, S)
    G32 = G.astype(np.float32)
    # exact per-core maxima for the fp8 scale choices
    gc_max = 1e-30
    vw_max = 1e-30
    ctx_max = float(np.abs(ctx_all).max())
    WV32 = WV64.astype(np.float32)
    for c in range(NCORES):
        gc_max = max(gc_max, float(np.abs(G32 @ ctx_all[c]).max()))
        vw_max = max(vw_max, float(np.abs(ctx_all[c].T @ WV32).max()))
    kc = int(np.floor(np.log2(200.0 / ctx_max)))
    kw = int(np.floor(np.log2(200.0 / max(float(np.abs(WV64).max()), 1e-30))))
    kg = int(np.floor(np.log2(200.0 / gc_max)))
    kv = int(np.floor(np.log2(200.0 / vw_max)))

    # TRN e4m3 tops out at +-240 (S.1111.000 is inf), so clip before casting.
    GT = np.clip(G.T * (2.0 ** kg), -240.0, 240.0).astype(np.float32)   # [e, c]
    # ct-major block permutation: A[p, ct*512+et*128+c'] = GT[et*128+p, ct*128+c']
    GT = np.ascontiguousarray(
        GT.reshape(4, 128, 4, 128).transpose(1, 2, 0, 3).reshape(128, 4 * C)
    ).astype(NP_F8)
    # wv et-major blocks: [p, et*E + o] = 2^kw WV[et*128+p, o]
    WVS = np.ascontiguousarray(
        np.clip(WV64 * (2.0 ** kw), -240.0, 240.0).astype(np.float32)
        .reshape(4, 128, E).transpose(1, 0, 2).reshape(128, 4 * E)
    ).astype(NP_F8)

    has_q0 = bool(np.any(q0_e))
    has_bo = bool(np.any(b_o))
    key = (has_q0, has_bo, kg, kc, kw, kv)
    if key not in _PROGRAM_CACHE:
        _PROGRAM_CACHE[key] = _build_program(has_q0, has_bo, kg, kc, kw, kv)
    nc = _PROGRAM_CACHE[key]

    in_maps = []
    for c in range(NCORES):
        ctx_mat = ctx_all[c]
        vbar = (ctx_mat.sum(axis=1).astype(np.float64) @ WV64) * (2.0 ** kv)
        # x permuted so chunk loads are contiguous per partition:
        # x_h[p, ic*2048 + t*512 + i] = x[t*128+p, ic*512+i]
        x8 = x[c].reshape(C, L).astype(NP_F8)
        x_h = np.ascontiguousarray(
            x8.reshape(4, 128, NCHUNK, LI).transpose(1, 2, 0, 3).reshape(128, 4 * L)
        )
        # ctx permuted jh-major then et-major:
        # ctx_h[p, jh*2048 + et*512 + j'] = (2^kc ctx)[et*128+p, jh*512+j']
        ctx8 = np.clip(ctx_mat * (2.0 ** kc), -240.0, 240.0).astype(NP_F8)
        ctx_h = np.ascontiguousarray(
            ctx8.reshape(4, 128, 2, 512).transpose(1, 2, 0, 3).reshape(128, 4 * S)
        )
        m = {
            "x": x_h,
            "ctx": ctx_h,
            "gt": GT,
            "wv": WVS,
            "vbar": np.ascontiguousarray(
                vbar.astype(np.float32).reshape(4, 128).T
            ),
        }
        if has_q0:
            # logits bias per key j: q0_e . ctx[:, j]  -> [S] -> [128, 8]
            q0j = (q0_e @ ctx_mat.astype(np.float64)).astype(np.float32)
            m["q0"] = np.ascontiguousarray(q0j.reshape(8, 128).T)
        if has_bo:
            m["bo"] = np.ascontiguousarray(b_o.reshape(4, 128).T)
        in_maps.append(m)

    res = run_bass_kernel_spmd(nc, in_maps, core_ids=list(range(NCORES)), trace=TRACE)
    LAST_RESULTS = res
    # y_h[p, ic*2048 + t*512 + i] = y[t*128+p, ic*512+i] -- invert the permute
    y = np.stack(
        [
            np.asarray(res.results[c]["y"]).astype(np.float32)
            .reshape(128, NCHUNK, 4, LI).transpose(2, 0, 1, 3).reshape(C, L)
            for c in range(NCORES)
        ],
        axis=0,
    )
    return np.ascontiguousarray(y.reshape(NCORES, C, 64, 64))


# revision 61
# speedup vs baseline: 30274.2071x; 1.0432x over previous
"""Cross-attention kernel for TRN2, data-parallel over batch (B=8) on 8 cores.

Reference computation per batch element:
    xt  = proj_in(x)              # [L=4096, E=512], 1x1 conv == matmul
    Q   = xt @ W_q.T + b_q
    K   = ctx @ W_k.T + b_k       # ctx: [S=1024, E]
    V   = ctx @ W_v.T + b_v
    att = softmax(Q @ K.T * scale)
    out = proj_out((att @ V).T)   # [C=512, 64, 64]

Host-side algebraic folds (weights only, exact up to fp rounding):
  * scale, W_pi, W_q, W_k fold into G = (scale * W_q @ W_pi).T @ W_k, so
    logits.T = (G.T-contract ctx).T-contract X.
  * W_v and W_po fold:  WV = (W_po @ W_v).T ; b_o = b_po + W_po @ b_v
  * Vbar[o] = sum_j VW[j, o] = (ctx.sum over keys) @ WV  -- the softmax
    mean-numerator -- is a tiny per-core [512] vector, computed on host.

The two big attention GEMMs (logits ST = GC.T-c X and output U = VW.T-c PT)
run as fp8e4m3 DoubleRow matmuls: 256 contraction rows per instruction --
2x the bf16 MAC rate.  fp8's 3-bit mantissa cannot represent softmax
weights P ~= 1 +- 0.1 (quantization step 0.125 at 1.0), so the softmax is
mean-split:  P = 1 + p,  U = Vbar + sum_j p_j VW_j with p = exp(s) - 1
cast to fp8 (full relative resolution on the deviation).  The Vbar mean
term and the invz division both fold into one DVE scalar_tensor_tensor:
y = (U_dev + Vbar) * invz.

Z = 1024 + sum_j p_j comes from a DoubleRow ones-matmul whose lhsT has 128
ones columns, so the psum holds 128 identical Z rows -- the reciprocal is
then already partition-replicated and no gpsimd broadcast is needed.

The per-core precomputes GC = G.T-c ctx and VW = ctx.T-c WV also run as
fp8 DoubleRow matmuls (gt/ctx/wv arrive as scaled e4m3); the psum->fp8
casts run on the Scalar engine (idle during that phase) with the inverse
input scales folded into the activation scale.  Scales kg/kv are chosen
from exact host-side maxima so GCD/VW8 use the full e4m3 normal range.

A burst of tiny warm-up matmuls on a memset tile runs during the initial
DMA wait so the PE's HAM clock gate (cold = 1.2 GHz, warm = 2.4 GHz) is
already released when the first real matmul issues.
"""

import numpy as np
import ml_dtypes

import concourse.bass as bass
import concourse.mybir as mybir
import concourse.tile as tile
from concourse import bacc
from concourse.bass_utils import run_bass_kernel_spmd

F32 = mybir.dt.float32
BF16 = mybir.dt.bfloat16
F8 = mybir.dt.float8e4
EXP = mybir.ActivationFunctionType.Exp
IDENT = mybir.ActivationFunctionType.Identity
COPY = mybir.ActivationFunctionType.Copy
DR = mybir.MatmulPerfMode.DoubleRow
AXX = mybir.AxisListType.X
ADD = mybir.AluOpType.add
MUL = mybir.AluOpType.mult

NP_F8 = ml_dtypes.float8_e4m3
NP_BF = ml_dtypes.bfloat16

C = 512       # in channels
E = 512       # emb dim
L = 4096      # query length (64*64)
S = 1024      # key length (32*32)
LI = 512      # i-chunk (query) tile size
NCHUNK = L // LI
NCORES = 8
NWARM = 30    # PE warm-up matmuls (N=128) during the DMA lead-in

TRACE = False           # test harness can flip this before calling kernel()
LAST_RESULTS = None     # stashed BassKernelResults for the test harness

_PROGRAM_CACHE = {}


def _two(ap, inner):
    """[128, 2*inner] AP -> [128, 2, inner] for DoubleRow operands."""
    return ap.rearrange("p (two n) -> p two n", two=2, n=inner)


def _build_program(has_q0: bool, has_bo: bool, kg: int, kc: int, kw: int, kv: int):
    nc = bacc.Bacc(
        "TRN2",
        target_bir_lowering=False,
        debug=False,
        enable_asserts=False,
        num_devices=NCORES,
    )
    # x host-permuted to x_d[p, ic*2048 + t*512 + i] = x[t*128+p, ic*512+i]
    # so each chunk load is one contiguous 2KB-per-partition DMA.
    x_d = nc.dram_tensor("x", [128, 4 * L], F8, kind="ExternalInput").ap()
    # ctx pre-scaled by 2^kc into e4m3 normal range, host-permuted et-major:
    # ctx_d[p, et*S + j] = (2^kc ctx)[et*128+p, j]
    ctx_d = nc.dram_tensor("ctx", [128, 4 * S], F8, kind="ExternalInput").ap()
    # gt arrives host-permuted into ct-major blocks: gt_d[p, ct*512+et*128+c']
    # = (2^kg * G.T)[et*128+p, ct*128+c'], e4m3.
    gt_d = nc.dram_tensor("gt", [128, 4 * C], F8, kind="ExternalInput").ap()
    # wv[p, et*E + o] = (2^kw * WV)[et*128+p, o], e4m3 (et-major blocks).
    wv_d = nc.dram_tensor("wv", [128, 4 * E], F8, kind="ExternalInput").ap()
    # vbar_d[p, ot] = (2^kv * Vbar)[ot*128+p]
    vbar_d = nc.dram_tensor("vbar", [128, 4], F32, kind="ExternalInput").ap()
    q0_d = bo_d = None
    if has_q0:
        q0_d = nc.dram_tensor("q0", [128, 8], F32, kind="ExternalInput").ap()
    if has_bo:
        bo_d = nc.dram_tensor("bo", [128, 4], F32, kind="ExternalInput").ap()
    # y device layout mirrors x: y_d[p, ic*2048 + t*512 + i] = y[t*128+p, ...]
    y_d = nc.dram_tensor("y", [128, 4 * L], BF16, kind="ExternalOutput").ap()

    exp_scale = float(2.0 ** -kg)
    gcd_scale = float(2.0 ** -kc)
    vw8_scale = float(2.0 ** (kv - kc - kw))
    zr_scale = float(2.0 ** kv)
    zr_bias = float(S) * zr_scale

    with tile.TileContext(nc) as tc:
        from contextlib import ExitStack

        with ExitStack() as ctx:
            cpool = ctx.enter_context(tc.tile_pool(name="consts", bufs=1))
            ps_s = ctx.enter_context(tc.tile_pool(name="ps_s", bufs=4, space="PSUM"))
            ps_z = ctx.enter_context(tc.tile_pool(name="ps_z", bufs=1, space="PSUM"))
            ps_u = ctx.enter_context(tc.tile_pool(name="ps_u", bufs=3, space="PSUM"))
            xpool = ctx.enter_context(tc.tile_pool(name="xp", bufs=2))
            ppool = ctx.enter_context(tc.tile_pool(name="pp", bufs=2))
            p8pool = ctx.enter_context(tc.tile_pool(name="p8p", bufs=2))
            opool = ctx.enter_context(tc.tile_pool(name="op", bufs=2))
            zpool = ctx.enter_context(tc.tile_pool(name="zp", bufs=2))

            # ---- PE warm-up: release the HAM clock gate during DMA wait ----
            # N=128 matmuls (~107ns cold) keep the PE duty cycle high enough
            # for the HAM activity window to flip to 2.4 GHz before the first
            # real matmul; N=32 warm-ups were too short to register as busy.
            # warm is memset on GpSimd, which is already active running the
            # framework's const-AP memsets -- the tile is ready before the
            # Tensor engine enters main, so warm-up starts immediately
            warm = cpool.tile([128, 128], BF16, name="warm")
            nc.gpsimd.memset(warm[:], 0.0)
            zrb = cpool.tile([128, 1], F32, name="zrb")
            nc.vector.memset(zrb[:], zr_bias)
            wps = ps_z.tile([128, 128], F32, name="wps", tag="z")
            for _ in range(NWARM):
                nc.tensor.matmul(wps[:], warm[:], warm[:], start=True, stop=True)

            # ---- loads in latency-priority order ---------------------------
            # (each dma_start costs ~0.7us of Sync-sequencer descriptor-gen
            # and all tensors are host-permuted to one contiguous descriptor
            # per partition row, so the preload is one DMA per tensor)
            # ctx is jh-major on host, so each half is contiguous and the
            # jh=0 GC groups are fully served by the first 256KB transfer.
            # (Splitting gt too starts GC ~0.4us earlier but starves the
            # GC/VW mid-phase -- measured net loss, so gt stays whole.)
            GTS = cpool.tile([128, 4 * C], F8, name="gstk", tag="gstk")
            CTXT = cpool.tile([128, 4 * S], F8, name="cstk", tag="cstk")
            nc.sync.dma_start(CTXT[:, 0:2 * S], ctx_d[:, 0:2 * S])
            nc.sync.dma_start(GTS[:], gt_d[:, :])
            nc.sync.dma_start(CTXT[:, 2 * S:4 * S], ctx_d[:, 2 * S:4 * S])
            WVT = cpool.tile([128, 4 * E], F8, name="wstk", tag="wstk")
            nc.sync.dma_start(WVT[:], wv_d[:, :])
            one8_s = cpool.tile([128, 256], F8, name="one8s")
            nc.vector.memset(one8_s[:], 1.0)

            def load_x(ic):
                xt = xpool.tile([128, 4 * LI], F8, name="xc", tag="x")
                nc.sync.dma_start(xt[:], x_d[:, bass.ts(ic, 4 * LI)])
                return xt

            vbar_s = cpool.tile([128, 4], F32, name="vbars")
            nc.sync.dma_start(vbar_s[:], vbar_d[:, :])
            X0 = load_x(0)                                        # prefetch chunk 0
            q0_s = bo_s = None
            if has_q0:
                q0_s = cpool.tile([128, 8], F32, name="q0s")
                nc.sync.dma_start(q0_s[:], q0_d[:, :])
            if has_bo:
                bo_s = cpool.tile([128, 4], F32, name="bos")
                nc.sync.dma_start(bo_s[:], bo_d[:, :])

            # ---- GC[c, j] = sum_e G[c, e] ctx[e, j]  (fp8 DR, once) -------
            # Output goes straight to the DoubleRow-interleaved fp8 layout:
            # GCD[cp][p, jt*256 + t*128 + m] = GC[(2cp+t)*128+p, jt*128+m],
            # scaled 2^kg (the 2^kc input scale divides out in the cast).
            GCD = [
                cpool.tile([128, 2048], F8, name=f"gcd{cp}", tag=f"gcd{cp}")
                for cp in range(2)
            ]
            for jh in range(2):
                # ctx_h[p, jh*2048 + et*512 + j'] = (2^kc ctx)[et*128+p, jh*512+j']
                ctxjh = CTXT[:, jh * 2 * S:(jh + 1) * 2 * S].rearrange(
                    "p (et j) -> p et j", et=4
                )
                for ct in range(4):
                    gps = ps_s.tile([128, LI], F32, name="gps", tag="s")
                    for eh in range(2):
                        nc.tensor.matmul(
                            gps[:],
                            _two(GTS[:, ct * 512 + eh * 256: ct * 512 + (eh + 1) * 256], 128),
                            ctxjh[:, 2 * eh:2 * eh + 2, :],
                            start=(eh == 0),
                            stop=(eh == 1),
                            perf_mode=DR,
                        )
                    dst = GCD[ct // 2][:, jh * 1024:(jh + 1) * 1024].rearrange(
                        "p (j two m) -> p j two m", two=2, m=128
                    )[:, :, ct % 2, :]
                    # alternate casts between Scalar and DVE: a single cast
                    # stream recycles the 4-deep psum pool slower than the
                    # matmuls fill it and throttles the whole GC/VW phase
                    if ct % 2 == 0:
                        nc.scalar.activation(
                            dst, gps[:].rearrange("p (j m) -> p j m", m=128),
                            COPY, scale=gcd_scale,
                        )
                    else:
                        nc.vector.tensor_scalar_mul(
                            dst, gps[:].rearrange("p (j m) -> p j m", m=128),
                            gcd_scale,
                        )

            # ---- VW[j, o] = sum_e ctx[e, j] WV[e, o]  (fp8 DR, once) ------
            # VW8D[jp][p, t*E + o] = 2^kv * VW[(2jp+t)*128+p, o]
            VW8D = [None] * 4

            def vw_group(jt):
                ctxjh = CTXT[:, (jt // 4) * 2 * S:(jt // 4 + 1) * 2 * S].rearrange(
                    "p (et j) -> p et j", et=4
                )
                js = jt % 4
                vps = ps_s.tile([128, E], F32, name="vps", tag="s")
                for eh in range(2):
                    nc.tensor.matmul(
                        vps[:],
                        ctxjh[:, 2 * eh:2 * eh + 2, js * 128:(js + 1) * 128],
                        _two(WVT[:, eh * 2 * E:(eh + 1) * 2 * E], E),
                        start=(eh == 0),
                        stop=(eh == 1),
                        perf_mode=DR,
                    )
                jp, t = jt // 2, jt % 2
                if t == 0:
                    VW8D[jp] = cpool.tile(
                        [128, 2 * E], F8, name=f"vw8_{jp}", tag=f"vw8_{jp}"
                    )
                if jt % 2 == 0:
                    nc.scalar.activation(
                        VW8D[jp][:, t * E:(t + 1) * E], vps[:], COPY,
                        scale=vw8_scale,
                    )
                else:
                    nc.vector.tensor_scalar_mul(
                        VW8D[jp][:, t * E:(t + 1) * E], vps[:], vw8_scale
                    )

            def st_group(ic, jt, X, pcur, p8cur):
                """ST[j,i] for one j-tile: 2 DoubleRow fp8 matmuls, exp on
                scalar (with the 2^-kg descale); after each jt-pair completes
                one DVE op casts p8 = P - 1 for the whole pair."""
                sps = ps_s.tile([128, LI], F32, name="sps", tag="s")
                nc.tensor.matmul(
                    sps[:],
                    _two(GCD[0][:, jt * 256:(jt + 1) * 256], 128),
                    _two(X[:, 0:2 * LI], LI),
                    start=True,
                    stop=False,
                    perf_mode=DR,
                )
                nc.tensor.matmul(
                    sps[:],
                    _two(GCD[1][:, jt * 256:(jt + 1) * 256], 128),
                    _two(X[:, 2 * LI:4 * LI], LI),
                    start=False,
                    stop=True,
                    perf_mode=DR,
                )
                jp, t = jt // 2, jt % 2
                if t == 0:
                    pcur[jp] = ppool.tile(
                        [128, 2 * LI], BF16, name=f"pt{jp}", tag=f"p{jp}"
                    )
                p = pcur[jp]
                if has_q0:
                    nc.scalar.activation(
                        p[:, t * LI:(t + 1) * LI], sps[:], EXP,
                        bias=q0_s[:, jt:jt + 1], scale=exp_scale,
                    )
                else:
                    nc.scalar.activation(
                        p[:, t * LI:(t + 1) * LI], sps[:], EXP, scale=exp_scale
                    )
                if t == 1:
                    p8cur[jp] = p8pool.tile(
                        [128, 2 * LI], F8, name=f"p8_{jp}", tag=f"p8_{jp}"
                    )
                    nc.vector.tensor_scalar_add(p8cur[jp][:], p[:], -1.0)

            def zsum_emit(p8prev):
                """2^kv * Z rows (all 128 partitions identical) via DoubleRow
                ones-matmul; zr = 2^kv*(1024 + sum p) on Scalar, reciprocal
                on DVE -- already partition-replicated, no broadcast."""
                zps = ps_z.tile([128, LI], F32, name="zps", tag="z")
                for jp in range(4):
                    nc.tensor.matmul(
                        zps[:],
                        _two(one8_s[:], 128),
                        _two(p8prev[jp][:], LI),
                        start=(jp == 0),
                        stop=(jp == 3),
                        perf_mode=DR,
                    )
                zr = zpool.tile([128, LI], F32, name="zr", tag="zr")
                nc.scalar.activation(zr[:], zps[:], IDENT, bias=zrb[:, 0:1], scale=zr_scale)
                invz = zpool.tile([128, LI], F32, name="invz", tag="invz")
                nc.vector.reciprocal_approx_fast(out=invz[:], in_=zr[:])
                return invz

            def u_group(ic, ot, p8prev, invz, ostate):
                """U_dev[o,i] = sum_j p_j VW[j,o] (psum, scaled 2^kv), then
                y = (U_dev + 2^kv Vbar) * invz on DVE (bf16 out).  The four
                ot tiles share one SBUF tile and ship as a single DMA."""
                if ic == NCHUNK - 1 and ot == 3:
                    # the Z bank is free after the final zsum; using it here
                    # avoids the last U group stalling on the ps_u rotation
                    ups = ps_z.tile([128, LI], F32, name="upz", tag="z")
                else:
                    ups = ps_u.tile([128, LI], F32, name="ups", tag="u")
                for jp in range(4):
                    nc.tensor.matmul(
                        ups[:],
                        _two(VW8D[jp][:], E)[:, :, ot * 128:(ot + 1) * 128],
                        _two(p8prev[jp][:], LI),
                        start=(jp == 0),
                        stop=(jp == 3),
                        perf_mode=DR,
                    )
                if ot == 0:
                    ostate["o"] = opool.tile([128, 4 * LI], BF16, name="ot", tag="o")
                o = ostate["o"]
                nc.vector.scalar_tensor_tensor(
                    o[:, ot * LI:(ot + 1) * LI], ups[:],
                    vbar_s[:, ot:ot + 1], invz[:], ADD, MUL,
                )
                if has_bo:
                    nc.vector.tensor_scalar_add(
                        o[:, ot * LI:(ot + 1) * LI],
                        o[:, ot * LI:(ot + 1) * LI], bo_s[:, ot:ot + 1],
                    )
                if ic == NCHUNK - 1:
                    # final chunk: ship each ot slice as soon as it is ready
                    # so the kernel's last DMA isn't gated on all four
                    nc.sync.dma_start(
                        y_d[:, ic * 4 * LI + ot * LI: ic * 4 * LI + (ot + 1) * LI],
                        o[:, ot * LI:(ot + 1) * LI],
                    )
                elif ot == 3:
                    nc.sync.dma_start(y_d[:, bass.ts(ic, 4 * LI)], o[:])

            # ---- window 0: ST(0) interleaved with the VW precompute -------
            # the exp pipeline starts while VW still owns the PE, so the
            # tail ST(0) groups aren't throttled to the Scalar exp rate
            X = X0
            pcur, p8cur = {}, {}
            st_group(0, 0, X, pcur, p8cur)
            st_group(0, 1, X, pcur, p8cur)
            for jt in range(8):
                vw_group(jt)
                if jt < 6:
                    st_group(0, jt + 2, X, pcur, p8cur)
            Xnext = load_x(1)

            # ---- windows 1..8: ST(w) interleaved with U(w-1) --------------
            invz_next = None
            for w in range(1, NCHUNK + 1):
                p8prev, p8cur = p8cur, {}
                pcur = {}
                X, Xnext = Xnext, (load_x(w + 1) if w + 1 < NCHUNK else None)
                invz = invz_next
                invz_next = None
                ostate = {}
                # in the final ST window, front-load the last jt pairs so the
                # final chunk's zsum (emitted at k=3 of THIS window, hidden
                # behind the U groups) isn't gated on the very last exp/cast
                pairs_at_k = [2, 4, 2, 0] if w == NCHUNK - 1 else [2, 2, 2, 2]
                jt_next = 0
                for k in range(4):
                    if w < NCHUNK:
                        for _ in range(pairs_at_k[k]):
                            st_group(w, jt_next, X, pcur, p8cur)
                            jt_next += 1
                    if k == 0 and invz is None:
                        invz = zsum_emit(p8prev)
                    if k == 3 and w == NCHUNK - 1:
                        # final chunk's Z overlaps the last U group of the
                        # previous chunk; its invz is ready when window
                        # NCHUNK's first U group stops
                        invz_next = zsum_emit(p8cur)
                    u_group(w - 1, k, p8prev, invz, ostate)

    nc.compile()
    return nc


def kernel(**inputs) -> np.ndarray:
    global LAST_RESULTS
    x = np.asarray(inputs["x"], dtype=np.float32)
    context = np.asarray(inputs["context"], dtype=np.float32)
    W_pi = np.asarray(inputs["W_pi"], dtype=np.float64)
    b_pi = np.asarray(inputs["b_pi"], dtype=np.float64)
    W_q = np.asarray(inputs["W_q"], dtype=np.float64)
    b_q = np.asarray(inputs["b_q"], dtype=np.float64)
    W_k = np.asarray(inputs["W_k"], dtype=np.float64)
    W_v = np.asarray(inputs["W_v"], dtype=np.float64)
    b_v = np.asarray(inputs["b_v"], dtype=np.float64)
    W_po = np.asarray(inputs["W_po"], dtype=np.float64)
    b_po = np.asarray(inputs["b_po"], dtype=np.float64)

    scale = float(E) ** -0.5
    Wqpi = scale * (W_q @ W_pi)                            # [dq, c]
    G = (Wqpi.T @ W_k)                                     # [c, e]
    b_row = scale * (W_q @ b_pi + b_q)
    q0_e = (W_k.T @ b_row).astype(np.float64)              # [e]
    WV64 = (W_po @ W_v).T                                  # [e, o]
    b_o = (b_po + W_po @ b_v).astype(np.float32)           # [o]

    ctx_all = context.reshape(NCORES, E, S)
    G32 = G.astype(np.float32)
    # exact per-core maxima for the fp8 scale choices
    gc_max = 1e-30
    vw_max = 1e-30
    ctx_max = float(np.abs(ctx_all).max())
    WV32 = WV64.astype(np.float32)
    for c in range(NCORES):
        gc_max = max(gc_max, float(np.abs(G32 @ ctx_all[c]).max()))
        vw_max = max(vw_max, float(np.abs(ctx_all[c].T @ WV32).max()))
    kc = int(np.floor(np.log2(200.0 / ctx_max)))
    kw = int(np.floor(np.log2(200.0 / max(float(np.abs(WV64).max()), 1e-30))))
    kg = int(np.floor(np.log2(200.0 / gc_max)))
    kv = int(np.floor(np.log2(200.0 / vw_max)))

    # TRN e4m3 tops out at +-240 (S.1111.000 is inf), so clip before casting.
    GT = np.clip(G.T * (2.0 ** kg), -240.0, 240.0).astype(np.float32)   # [e, c]
    # ct-major block permutation: A[p, ct*512+et*128+c'] = GT[et*128+p, ct*128+c']
    GT = np.ascontiguousarray(
        GT.reshape(4, 128, 4, 128).transpose(1, 2, 0, 3).reshape(128, 4 * C)
    ).astype(NP_F8)
    # wv et-major blocks: [p, et*E + o] = 2^kw WV[et*128+p, o]
    WVS = np.ascontiguousarray(
        np.clip(WV64 * (2.0 ** kw), -240.0, 240.0).astype(np.float32)
        .reshape(4, 128, E).transpose(1, 0, 2).reshape(128, 4 * E)
    ).astype(NP_F8)

    has_q0 = bool(np.any(q0_e))
    has_bo = bool(np.any(b_o))
    key = (has_q0, has_bo, kg, kc, kw, kv)
    if key not in _PROGRAM_CACHE:
        _PROGRAM_CACHE[key] = _build_program(has_q0, has_bo, kg, kc, kw, kv)
    nc = _PROGRAM_CACHE[key]

    in_maps = []
    for c in range(NCORES):
        ctx_mat = ctx_all[c]
        vbar = (ctx_mat.sum(axis=1).astype(np.float64) @ WV64) * (2.0 ** kv)
        # x permuted so chunk loads are contiguous per partition:
        # x_h[p, ic*2048 + t*512 + i] = x[t*128+p, ic*512+i]
        x8 = x[c].reshape(C, L).astype(NP_F8)
        x_h = np.ascontiguousarray(
            x8.reshape(4, 128, NCHUNK, LI).transpose(1, 2, 0, 3).reshape(128, 4 * L)
        )
        # ctx permuted jh-major then et-major:
        # ctx_h[p, jh*2048 + et*512 + j'] = (2^kc ctx)[et*128+p, jh*512+j']
        ctx8 = np.clip(ctx_mat * (2.0 ** kc), -240.0, 240.0).astype(NP_F8)
        ctx_h = np.ascontiguousarray(
            ctx8.reshape(4, 128, 2, 512).transpose(1, 2, 0, 3).reshape(128, 4 * S)
        )
        m = {
            "x": x_h,
            "ctx": ctx_h,
            "gt": GT,
            "wv": WVS,
            "vbar": np.ascontiguousarray(
                vbar.astype(np.float32).reshape(4, 128).T
            ),
        }
        if has_q0:
            # logits bias per key j: q0_e . ctx[:, j]  -> [S] -> [128, 8]
            q0j = (q0_e @ ctx_mat.astype(np.float64)).astype(np.float32)
            m["q0"] = np.ascontiguousarray(q0j.reshape(8, 128).T)
        if has_bo:
            m["bo"] = np.ascontiguousarray(b_o.reshape(4, 128).T)
        in_maps.append(m)

    res = run_bass_kernel_spmd(nc, in_maps, core_ids=list(range(NCORES)), trace=TRACE)
    LAST_RESULTS = res
    # y_h[p, ic*2048 + t*512 + i] = y[t*128+p, ic*512+i] -- invert the permute
    y = np.stack(
        [
            np.asarray(res.results[c]["y"]).astype(np.float32)
            .reshape(128, NCHUNK, 4, LI).transpose(2, 0, 1, 3).reshape(C, L)
            for c in range(NCORES)
        ],
        axis=0,
    )
    return np.ascontiguousarray(y.reshape(NCORES, C, 64, 64))
